# revision 19
# baseline (speedup 1.0000x reference)
"""CALoraLinear kernel for 8 TRN2 NeuronCores (Bass/Tile, SPMD).

Math (derived from the reference):
  orig = x @ W.T + bias
  top2 classes c1,c2 per row from pseudo_index[b, :64]
  g_j = <lora_A[c_j], x[b]>          (only rows 0..63 of lora_A are reachable)
  lora_out[b,o] = 16 * sum_c mask[b,c] * G[b,c] * lora_B[o,c]
  out = orig + lora_out + bias       (bias added twice)

Sharding: column-shard W across the 8 cores (each core owns 512 output
columns, full batch); x / lora_A / pseudo_index replicated. Host
concatenates the per-core [512, 512] blocks along the output axis.

v2 design (PE-bound; DMA measured at ~420 GB/s/core sustained):
  - f16 streams for x and W (PE streams 1 col/cycle; fp8 would need
    error compensation that costs more than it saves).
  - lora_A + the small pack (pseudo_index layouts, scaled lora_B, bias)
    are loaded once up front; only x|W stream per chunk. First chunks
    are small (2 k-tiles) to cut time-to-first-matmul, then 4 k-tiles.
  - A dummy-matmul warmup train runs during the DMA preamble so the PE
    HAM clock gate is already at 8/8 when real data lands.
  - G matmuls are issued as ADJACENT column-tile pairs (even k -> array
    cols 0-63, odd k -> 64-127) so each pair streams concurrently:
    512 cycles per 2 k-tiles instead of 1024.
  - Threshold/mask pipeline (top-2 over 64 classes) runs on DVE/SWDGE
    entirely under the main stream.
  - Tail is pipelined per batch-tile: gts+mask -> f32r tail matmul into
    the main PSUM bank -> PSUM->SBUF copy (alternating Vector/Scalar
    engines) -> f16 output DMA (host upcasts to f32).
"""

import os
import sys

for _p in ("/opt/trn_rl_repo",):
    if _p not in sys.path:
        sys.path.insert(0, _p)

import numpy as np

import concourse.bass as bass
import concourse.bacc as bacc
import concourse.mybir as mybir
from concourse.tile import TileContext, add_dep_helper
from concourse.bass_utils import run_bass_kernel_spmd


def _ensure_ntff_hook_module():
    """run_bass_kernel_spmd(trace=True) imports antenv.axon_hooks, which the
    agent image's antenv package lacks. Provide it (and register the real
    ctypes NTFF hook when available) so a tracing caller doesn't crash."""
    import types

    try:
        import antenv
    except ImportError:
        return
    if getattr(antenv, "axon_hooks", None) is not None:
        return
    mod = types.ModuleType("antenv.axon_hooks")
    state = {"hook": None}
    mod.set_axon_ntff_profile_hook = lambda h: state.__setitem__("hook", h)
    mod.get_axon_ntff_profile_hook = lambda: state["hook"]
    sys.modules["antenv.axon_hooks"] = mod
    antenv.axon_hooks = mod
    try:
        from trn_agent_boot.trn_boot import _ntff_profile_via_ctypes

        mod.set_axon_ntff_profile_hook(
            _ntff_profile_via_ctypes("/opt/axon/libaxon_pjrt.so")
        )
    except Exception:
        pass


_ensure_ntff_hook_module()

B, IN, OUT = 512, 4096, 4096
NUM_CLASS, RANK = 64, 8
NCORES = 8
OUT_L = OUT // NCORES  # 512
P = 128
KT = IN // P           # 32 k-tiles
BT = B // P            # 4 batch tiles

# chunk schedule: k-tiles per streamed chunk (first ones small to cut
# time-to-first-matmul and ride out the DMA ramp, last ones small to
# shrink the end-of-stream residual work)
CHUNK_KS = [1, 1, 2, 2, 2, 2, 4, 4, 4, 4, 4, 2]
assert sum(CHUNK_KS) == KT
NCHUNK = len(CHUNK_KS)

# column layout inside one chunk of ck k-tiles: [x: ck*B][w: ck*OUT_L]
def _chunk_width(ck):
    return ck * (B + OUT_L)

XW_WIDTH = max(_chunk_width(ck) for ck in CHUNK_KS)  # 4096

# pp layout: [ps: BT*64][psT: B]
PSOFF = 0
PTOFF = BT * NUM_CLASS          # 256
PPW = PTOFF + B                 # 768

# av layout (f16): [lora_A chunks: KT*64][bS: OUT_L (rows 0:65)]
AW = KT * NUM_CLASS             # 2048
AVW = AW + OUT_L                # 2560
# av is DMA'd in two pieces: head (lora_A for k<16, needed early for the
# first G matmuls) and tail (rest of lora_A + bS, needed later)
AHW = (KT // 2) * NUM_CLASS     # 1024

F32 = mybir.dt.float32
F32R = mybir.dt.float32r
F16 = mybir.dt.float16
X = mybir.AxisListType.X

NWARM = 96  # dummy warmup matmuls (N=64 each, ~55ns cold) ~ 5.4us; must
            # bridge the PE from kernel start to first-chunk arrival so
            # the HAM clock-gate stays at 8/8 for the real matmuls

_cache = {}
# test.py reads this after a traced run for HW exec time
last_results = None


def _build():
    key = "nc_v2"
    if key in _cache:
        return _cache[key]
    nc = bacc.Bacc(
        bass.get_trn_type() or "TRN2",
        target_bir_lowering=False,
        debug=False,
        num_devices=NCORES,
    )

    xw = nc.dram_tensor("xw", [NCHUNK, P, XW_WIDTH], F16, kind="ExternalInput")
    av = nc.dram_tensor("av", [P, AVW], F16, kind="ExternalInput")
    pp = nc.dram_tensor("pp", [P, PPW], F32R, kind="ExternalInput")
    out = nc.dram_tensor("out", [B, OUT_L], F16, kind="ExternalOutput")

    with TileContext(nc) as tc:
        with (
            tc.tile_pool(name="xwp", bufs=1) as xwpool,
            tc.tile_pool(name="sml", bufs=1) as spool,
            tc.tile_pool(name="tl", bufs=1) as tpool,
            tc.tile_pool(name="op", bufs=1) as opool,
            tc.tile_pool(name="dr", bufs=1, space="DRAM") as dpool,
            tc.tile_pool(name="ps", bufs=1, space="PSUM") as ppool,
        ):
            # ---- input DMAs, issued first. Each HWDGE queue serves its
            # DMAs FIFO at ~half the aggregate rate, so each queue's order
            # must match PE consumption order: sync = even chunks (+ the
            # output writes later), scalar = lora_A then odd chunks, and
            # pp rides the SWDGE (gpsimd) queue — it only feeds the
            # threshold pipeline, which is off the critical path. ----
            chunk_tiles = []
            # c0 rides the SWDGE (gpsimd) queue so all three queues start
            # delivering chunks immediately; c1 leads the scalar queue and
            # c2 the sync queue.
            c0 = xwpool.tile([P, _chunk_width(CHUNK_KS[0])], F16, tag="xwc0",
                             name="xwc0")
            nc.gpsimd.dma_start(out=c0, in_=xw[0, :, : _chunk_width(CHUNK_KS[0])])
            chunk_tiles.append(c0)

            c1 = xwpool.tile([P, _chunk_width(CHUNK_KS[1])], F16, tag="xwc1",
                             name="xwc1")
            nc.scalar.dma_start(out=c1, in_=xw[1, :, : _chunk_width(CHUNK_KS[1])])
            chunk_tiles.append(c1)

            av_sb = spool.tile([P, AVW], F16)
            nc.scalar.dma_start(out=av_sb[:, :AHW], in_=av[:, :AHW])
            pp_sb = spool.tile([P, PPW], F32R)
            nc.gpsimd.dma_start(out=pp_sb, in_=pp[:, :])

            a_sb = av_sb[:, :AW]
            bS_sb = av_sb[: NUM_CLASS + 1, AW:AVW]

            av_tail_at = NCHUNK // 2  # dispatch position for av tail piece
            for c in range(2, NCHUNK):
                w = _chunk_width(CHUNK_KS[c])
                t = xwpool.tile([P, w], F16, tag=f"xwc{c}", name=f"xwc{c}")
                eng = nc.sync if c % 2 == 0 else nc.scalar
                if c == av_tail_at:
                    nc.scalar.dma_start(out=av_sb[:, AHW:], in_=av[:, AHW:])
                dma = eng.dma_start(out=t, in_=xw[c, :, :w])
                chunk_tiles.append(t)

            ps_sb = pp_sb[:, PSOFF : PSOFF + BT * NUM_CLASS].bitcast(F32)
            psT_sb = pp_sb[:NUM_CLASS, PTOFF : PTOFF + B].bitcast(F32)

            # ---- PE warmup train: keep the HAM clock-gate busy while the
            # first chunks stream in, so real matmuls start at 2.4 GHz ----
            warm_sb = spool.tile([P, NUM_CLASS], F16)
            nc.vector.memset(warm_sb, 0.0)
            warm_ps = ppool.tile([NUM_CLASS, NUM_CLASS], F32, tag="warm",
                                 name="warm_ps")
            for _ in range(NWARM):
                nc.tensor.matmul(
                    warm_ps, lhsT=warm_sb, rhs=warm_sb[:, :NUM_CLASS],
                    start=True, stop=True,
                )

            # ---- top-2 threshold per batch row (DVE, under the stream) ----
            m2col = spool.tile([P, BT], F32)
            for bt in range(BT):
                pt = ps_sb[:, bt * NUM_CLASS : (bt + 1) * NUM_CLASS]
                m1 = spool.tile([P, 1], F32, tag=f"m1_{bt}")
                nc.vector.reduce_max(out=m1, in_=pt, axis=X)
                negmask = spool.tile([P, NUM_CLASS], F32, tag=f"nm_{bt}")
                # (pt >= m1) * -1e30  -> additive mask that kills the max
                nc.vector.tensor_scalar(
                    out=negmask,
                    in0=pt,
                    scalar1=m1,
                    scalar2=-1.0e30,
                    op0=mybir.AluOpType.is_ge,
                    op1=mybir.AluOpType.mult,
                )
                p2 = spool.tile([P, NUM_CLASS], F32, tag=f"p2_{bt}")
                nc.vector.tensor_tensor(
                    out=p2, in0=pt, in1=negmask, op=mybir.AluOpType.add
                )
                nc.vector.reduce_max(out=m2col[:, bt : bt + 1], in_=p2, axis=X)

            # threshold shuffle partition->free via a DRAM bounce on the
            # GPSIMD (SWDGE) path, concurrent with the HWDGE chunk stream
            m2d = dpool.tile([BT, P], F32)
            nc.gpsimd.dma_start(out=m2d.rearrange("bt p -> p bt"), in_=m2col[:, :])
            thr_sb = spool.tile([NUM_CLASS, B], F32)
            nc.gpsimd.dma_start(
                out=thr_sb,
                in_=m2d.rearrange("bt p -> (bt p)")[None, :].broadcast_to(
                    [NUM_CLASS, B]
                ),
            )
            maskT = tpool.tile([NUM_CLASS, B], F32)
            nc.vector.tensor_tensor(
                out=maskT, in0=psT_sb, in1=thr_sb, op=mybir.AluOpType.is_ge
            )
            # ht: [mask*G rows 0:64][ones row 64]; ones row built early
            ht = tpool.tile([NUM_CLASS + 1, B], F16)
            nc.vector.tensor_scalar(
                out=ht[NUM_CLASS : NUM_CLASS + 1, :],
                in0=thr_sb[0:1, :],
                scalar1=0.0,
                scalar2=1.0,
                op0=mybir.AluOpType.mult,
                op1=mybir.AluOpType.add,
            )

            # ---- PSUM accumulators ----
            mps = [
                ppool.tile([P, OUT_L], F32, tag=f"main{bt}", name=f"main{bt}")
                for bt in range(BT)
            ]
            # G accumulates as two concurrent column-tiles (even k -> array
            # cols 0-63, odd k -> 64-127); issuing the two halves adjacently
            # makes them stream concurrently. The halves live in SEPARATE
            # PSUM banks so the even half's mask-multiply can run while the
            # odd half is still being written by the PE.
            gt0_ps = ppool.tile([2 * NUM_CLASS, B], F32, tag="gt0", name="gt0_ps")
            gt1_ps = ppool.tile([2 * NUM_CLASS, B], F32, tag="gt1", name="gt1_ps")

            def g_mm(k, xk):
                half = k % 2
                gt = gt0_ps if half == 0 else gt1_ps
                nc.tensor.matmul(
                    gt[half * NUM_CLASS : (half + 1) * NUM_CLASS, :],
                    lhsT=a_sb[:, k * NUM_CLASS : (k + 1) * NUM_CLASS],
                    rhs=xk,
                    start=(k == half),
                    stop=(k == KT - 2 + half),
                    tile_position=(0, half * NUM_CLASS),
                )

            def main_mms(k, xk, wk):
                for bt in range(BT):
                    nc.tensor.matmul(
                        mps[bt],
                        lhsT=xk[:, bt * P : (bt + 1) * P],
                        rhs=wk,
                        start=(k == 0),
                        stop=False,
                    )

            # ---- main streaming loop ----
            # mains for pairs of k, then the G pair adjacently so the two
            # column-tiles overlap on the array. For the final pair issue
            # the G matmuls before the last mains so the tail DVE chain
            # overlaps them.
            kslices = []
            for c, ck in enumerate(CHUNK_KS):
                t = chunk_tiles[c]
                woff = ck * B
                for kk in range(ck):
                    kslices.append(
                        (
                            t[:, kk * B : (kk + 1) * B],
                            t[:, woff + kk * OUT_L : woff + (kk + 1) * OUT_L],
                        )
                    )
            for k0 in range(0, KT, 2):
                (x0, w0), (x1, w1) = kslices[k0], kslices[k0 + 1]
                if k0 == KT - 2:
                    main_mms(k0, x0, w0)
                    g_mm(k0, x0)
                    g_mm(k0 + 1, x1)
                    main_mms(k0 + 1, x1, w1)
                else:
                    main_mms(k0, x0, w0)
                    main_mms(k0 + 1, x1, w1)
                    g_mm(k0, x0)
                    g_mm(k0 + 1, x1)

            # ---- tail ----
            # mask-multiply each G column-half straight out of PSUM (one
            # PSUM input per DVE op), merge into ht (f16), then the four
            # f16 tail matmuls back-to-back; copies alternate DVE/ACT and
            # the output DMAs alternate sync/gpsimd so nothing serializes
            # on a single engine.
            o_all = opool.tile([P, BT * OUT_L], F16)
            h0 = tpool.tile([NUM_CLASS, B], F32)
            h1 = tpool.tile([NUM_CLASS, B], F32)
            # h0 depends only on the even G half (stops at k=30, before the
            # final mains), so it overlaps the end of the stream
            nc.vector.tensor_tensor(
                out=h0, in0=gt0_ps[0:NUM_CLASS, :], in1=maskT,
                op=mybir.AluOpType.mult,
            )
            nc.vector.tensor_tensor(
                out=h1, in0=gt1_ps[NUM_CLASS : 2 * NUM_CLASS, :], in1=maskT,
                op=mybir.AluOpType.mult,
            )
            nc.vector.tensor_tensor(
                out=ht[0:NUM_CLASS, :], in0=h0, in1=h1, op=mybir.AluOpType.add
            )
            for bt in range(BT):
                sl = slice(bt * P, (bt + 1) * P)
                nc.tensor.matmul(
                    mps[bt],
                    lhsT=ht[:, sl],
                    rhs=bS_sb,
                    start=False,
                    stop=True,
                )
            for bt in range(BT):
                sl = slice(bt * P, (bt + 1) * P)
                osl = o_all[:, bt * OUT_L : (bt + 1) * OUT_L]
                if bt % 2 == 0:
                    nc.vector.tensor_copy(out=osl, in_=mps[bt])
                    nc.sync.dma_start(out=out[sl, :], in_=osl)
                else:
                    nc.scalar.copy(out=osl, in_=mps[bt])
                    nc.scalar.dma_start(out=out[sl, :], in_=osl)

    nc.finalize()
    _cache[key] = nc
    return nc


def _pack_inputs(x, pseudo_index, weight, bias, lora_A, lora_B):
    """Build the interleaved per-core xw chunk buffers + replicated small
    inputs (lora_A chunks, pseudo_index layouts, scaled lora_B/bias)."""
    xT = np.ascontiguousarray(x.T).astype(np.float16)   # [IN, B]
    aT = lora_A[:NUM_CLASS].T.astype(np.float16)        # [IN, 64]

    # av: [128, AVW]: [p, k*64+c] = aT[k*128+p, c], then bS (per-core)
    av_base = np.zeros((P, AVW), dtype=np.float16)
    av_base[:, :AW] = (
        aT.reshape(KT, P, NUM_CLASS).transpose(1, 0, 2).reshape(P, KT * NUM_CLASS)
    )

    pp_base = np.zeros((P, PPW), dtype=np.float32)
    pp_base[:, PSOFF : PSOFF + BT * NUM_CLASS] = (
        pseudo_index.reshape(BT, P, NUM_CLASS)
        .transpose(1, 0, 2)
        .reshape(P, BT * NUM_CLASS)
    )
    pp_base[:NUM_CLASS, PTOFF : PTOFF + B] = pseudo_index.T

    # per-chunk x blocks: for chunk c with k-tiles [k0, k0+ck):
    #   [p, kk*B + b] = xT[(k0+kk)*P + p, b]
    x3 = xT.reshape(KT, P, B)  # [k, p, b]

    in_maps = []
    for i in range(NCORES):
        o0 = i * OUT_L
        wTi = weight[o0 : o0 + OUT_L].T.astype(np.float16)  # [IN, OUT_L]
        w3 = wTi.reshape(KT, P, OUT_L)
        xwi = np.zeros((NCHUNK, P, XW_WIDTH), dtype=np.float16)
        k0 = 0
        for c, ck in enumerate(CHUNK_KS):
            xwi[c, :, : ck * B] = (
                x3[k0 : k0 + ck].transpose(1, 0, 2).reshape(P, ck * B)
            )
            xwi[c, :, ck * B : ck * (B + OUT_L)] = (
                w3[k0 : k0 + ck].transpose(1, 0, 2).reshape(P, ck * OUT_L)
            )
            k0 += ck
        avi = av_base.copy()
        avi[:NUM_CLASS, AW:AVW] = 16.0 * lora_B[o0 : o0 + OUT_L, :NUM_CLASS].T
        avi[NUM_CLASS, AW:AVW] = 2.0 * bias[o0 : o0 + OUT_L]
        in_maps.append({"xw": xwi, "av": avi, "pp": pp_base})
    return in_maps


def kernel(x, pseudo_index, weight, bias, lora_A, lora_B):
    global last_results
    x = np.ascontiguousarray(np.asarray(x, dtype=np.float32))
    pseudo_index = np.ascontiguousarray(np.asarray(pseudo_index, dtype=np.float32))
    weight = np.asarray(weight, dtype=np.float32)
    bias = np.asarray(bias, dtype=np.float32)
    lora_A = np.asarray(lora_A, dtype=np.float32)
    lora_B = np.asarray(lora_B, dtype=np.float32)

    nc = _build()
    in_maps = _pack_inputs(x, pseudo_index, weight, bias, lora_A, lora_B)
    res = run_bass_kernel_spmd(nc, in_maps, list(range(NCORES)))
    last_results = res
    return np.hstack(
        [res.results[i]["out"].astype(np.float32) for i in range(NCORES)]
    )


# revision 20
# speedup vs baseline: 1.0824x; 1.0824x over previous
"""CALoraLinear kernel for 8 TRN2 NeuronCores (Bass/Tile, SPMD).

Math (derived from the reference):
  orig = x @ W.T + bias
  top2 classes c1,c2 per row from pseudo_index[b, :64]
  g_j = <lora_A[c_j], x[b]>          (only rows 0..63 of lora_A are reachable)
  lora_out[b,o] = 16 * sum_c mask[b,c] * G[b,c] * lora_B[o,c]
  out = orig + lora_out + bias       (bias added twice)

Sharding: column-shard W across the 8 cores (each core owns 512 output
columns, full batch); x / lora_A / pseudo_index replicated. Host
concatenates the per-core [512, 512] blocks along the output axis.

v2 design (PE-bound; DMA measured at ~420 GB/s/core sustained):
  - f16 streams for x and W (PE streams 1 col/cycle; fp8 would need
    error compensation that costs more than it saves).
  - lora_A + the small pack (pseudo_index layouts, scaled lora_B, bias)
    are loaded once up front; only x|W stream per chunk. First chunks
    are small (2 k-tiles) to cut time-to-first-matmul, then 4 k-tiles.
  - A dummy-matmul warmup train runs during the DMA preamble so the PE
    HAM clock gate is already at 8/8 when real data lands.
  - G matmuls are issued as ADJACENT column-tile pairs (even k -> array
    cols 0-63, odd k -> 64-127) so each pair streams concurrently:
    512 cycles per 2 k-tiles instead of 1024.
  - Threshold/mask pipeline (top-2 over 64 classes) runs on DVE/SWDGE
    entirely under the main stream.
  - Tail is pipelined per batch-tile: gts+mask -> f32r tail matmul into
    the main PSUM bank -> PSUM->SBUF copy (alternating Vector/Scalar
    engines) -> f16 output DMA (host upcasts to f32).
"""

import os
import sys

for _p in ("/opt/trn_rl_repo",):
    if _p not in sys.path:
        sys.path.insert(0, _p)

import numpy as np

import concourse.bass as bass
import concourse.bacc as bacc
import concourse.mybir as mybir
from concourse.tile import TileContext, add_dep_helper
from concourse.bass_utils import run_bass_kernel_spmd


def _ensure_ntff_hook_module():
    """run_bass_kernel_spmd(trace=True) imports antenv.axon_hooks, which the
    agent image's antenv package lacks. Provide it (and register the real
    ctypes NTFF hook when available) so a tracing caller doesn't crash."""
    import types

    try:
        import antenv
    except ImportError:
        return
    if getattr(antenv, "axon_hooks", None) is not None:
        return
    mod = types.ModuleType("antenv.axon_hooks")
    state = {"hook": None}
    mod.set_axon_ntff_profile_hook = lambda h: state.__setitem__("hook", h)
    mod.get_axon_ntff_profile_hook = lambda: state["hook"]
    sys.modules["antenv.axon_hooks"] = mod
    antenv.axon_hooks = mod
    try:
        from trn_agent_boot.trn_boot import _ntff_profile_via_ctypes

        mod.set_axon_ntff_profile_hook(
            _ntff_profile_via_ctypes("/opt/axon/libaxon_pjrt.so")
        )
    except Exception:
        pass


_ensure_ntff_hook_module()

B, IN, OUT = 512, 4096, 4096
NUM_CLASS, RANK = 64, 8
NCORES = 8
OUT_L = OUT // NCORES  # 512
P = 128
KT = IN // P           # 32 k-tiles
BT = B // P            # 4 batch tiles

# chunk schedule: k-tiles per streamed chunk (first ones small to cut
# time-to-first-matmul and ride out the DMA ramp, last ones small to
# shrink the end-of-stream residual work)
CHUNK_KS = [1, 1, 2, 2, 2, 2, 4, 4, 4, 4, 4, 2]
assert sum(CHUNK_KS) == KT
NCHUNK = len(CHUNK_KS)

# column layout inside one chunk of ck k-tiles: [x: ck*B][w: ck*OUT_L]
def _chunk_width(ck):
    return ck * (B + OUT_L)

XW_WIDTH = max(_chunk_width(ck) for ck in CHUNK_KS)  # 4096

# pp layout: [ps: BT*64][psT: B]
PSOFF = 0
PTOFF = BT * NUM_CLASS          # 256
PPW = PTOFF + B                 # 768

# av layout (f16): [lora_A chunks: KT*64][bS: OUT_L (rows 0:65)]
AW = KT * NUM_CLASS             # 2048
AVW = AW + OUT_L                # 2560
# av is DMA'd in two pieces: head (lora_A for k<16, needed early for the
# first G matmuls) and tail (rest of lora_A + bS, needed later)
AHW = (KT // 2) * NUM_CLASS     # 1024

F32 = mybir.dt.float32
F32R = mybir.dt.float32r
F16 = mybir.dt.float16
X = mybir.AxisListType.X

NWARM = 96  # dummy warmup matmuls (N=64 each, ~55ns cold) ~ 5.4us; must
            # bridge the PE from kernel start to first-chunk arrival so
            # the HAM clock-gate stays at 8/8 for the real matmuls

_cache = {}
# test.py reads this after a traced run for HW exec time
last_results = None


def _build():
    key = "nc_v2"
    if key in _cache:
        return _cache[key]
    nc = bacc.Bacc(
        bass.get_trn_type() or "TRN2",
        target_bir_lowering=False,
        debug=False,
        num_devices=NCORES,
    )

    xw = nc.dram_tensor("xw", [NCHUNK, P, XW_WIDTH], F16, kind="ExternalInput")
    av = nc.dram_tensor("av", [P, AVW], F16, kind="ExternalInput")
    pp = nc.dram_tensor("pp", [P, PPW], F32R, kind="ExternalInput")
    out = nc.dram_tensor("out", [B, OUT_L], F16, kind="ExternalOutput")

    with TileContext(nc) as tc:
        with (
            tc.tile_pool(name="xwp", bufs=1) as xwpool,
            tc.tile_pool(name="sml", bufs=1) as spool,
            tc.tile_pool(name="tl", bufs=1) as tpool,
            tc.tile_pool(name="op", bufs=1) as opool,
            tc.tile_pool(name="dr", bufs=1, space="DRAM") as dpool,
            tc.tile_pool(name="ps", bufs=1, space="PSUM") as ppool,
        ):
            # ---- input DMAs, issued first. Each HWDGE queue serves its
            # DMAs FIFO at ~half the aggregate rate, so each queue's order
            # must match PE consumption order: sync = even chunks (+ the
            # output writes later), scalar = lora_A then odd chunks, and
            # pp rides the SWDGE (gpsimd) queue — it only feeds the
            # threshold pipeline, which is off the critical path. ----
            chunk_tiles = []
            # c0 leads the sync queue, c1 the scalar queue; pp rides the
            # SWDGE (gpsimd) queue which only feeds the off-critical-path
            # threshold pipeline.
            c0 = xwpool.tile([P, _chunk_width(CHUNK_KS[0])], F16, tag="xwc0",
                             name="xwc0")
            nc.sync.dma_start(out=c0, in_=xw[0, :, : _chunk_width(CHUNK_KS[0])])
            chunk_tiles.append(c0)

            c1 = xwpool.tile([P, _chunk_width(CHUNK_KS[1])], F16, tag="xwc1",
                             name="xwc1")
            nc.scalar.dma_start(out=c1, in_=xw[1, :, : _chunk_width(CHUNK_KS[1])])
            chunk_tiles.append(c1)

            av_sb = spool.tile([P, AVW], F16)
            nc.scalar.dma_start(out=av_sb[:, :AHW], in_=av[:, :AHW])
            pp_sb = spool.tile([P, PPW], F32R)
            nc.gpsimd.dma_start(out=pp_sb, in_=pp[:, :])

            a_sb = av_sb[:, :AW]
            bS_sb = av_sb[: NUM_CLASS + 1, AW:AVW]

            av_tail_at = NCHUNK // 2  # dispatch position for av tail piece
            for c in range(2, NCHUNK):
                w = _chunk_width(CHUNK_KS[c])
                t = xwpool.tile([P, w], F16, tag=f"xwc{c}", name=f"xwc{c}")
                eng = nc.sync if c % 2 == 0 else nc.scalar
                if c == av_tail_at:
                    nc.scalar.dma_start(out=av_sb[:, AHW:], in_=av[:, AHW:])
                dma = eng.dma_start(out=t, in_=xw[c, :, :w])
                chunk_tiles.append(t)

            ps_sb = pp_sb[:, PSOFF : PSOFF + BT * NUM_CLASS].bitcast(F32)
            psT_sb = pp_sb[:NUM_CLASS, PTOFF : PTOFF + B].bitcast(F32)

            # ---- PE warmup train: keep the HAM clock-gate busy while the
            # first chunks stream in, so real matmuls start at 2.4 GHz ----
            warm_sb = spool.tile([P, NUM_CLASS], F16)
            nc.vector.memset(warm_sb, 0.0)
            warm_ps = ppool.tile([NUM_CLASS, NUM_CLASS], F32, tag="warm",
                                 name="warm_ps")
            for _ in range(NWARM):
                nc.tensor.matmul(
                    warm_ps, lhsT=warm_sb, rhs=warm_sb[:, :NUM_CLASS],
                    start=True, stop=True,
                )

            # ---- top-2 threshold per batch row (DVE, under the stream) ----
            m2col = spool.tile([P, BT], F32)
            for bt in range(BT):
                pt = ps_sb[:, bt * NUM_CLASS : (bt + 1) * NUM_CLASS]
                m1 = spool.tile([P, 1], F32, tag=f"m1_{bt}")
                nc.vector.reduce_max(out=m1, in_=pt, axis=X)
                negmask = spool.tile([P, NUM_CLASS], F32, tag=f"nm_{bt}")
                # (pt >= m1) * -1e30  -> additive mask that kills the max
                nc.vector.tensor_scalar(
                    out=negmask,
                    in0=pt,
                    scalar1=m1,
                    scalar2=-1.0e30,
                    op0=mybir.AluOpType.is_ge,
                    op1=mybir.AluOpType.mult,
                )
                p2 = spool.tile([P, NUM_CLASS], F32, tag=f"p2_{bt}")
                nc.vector.tensor_tensor(
                    out=p2, in0=pt, in1=negmask, op=mybir.AluOpType.add
                )
                nc.vector.reduce_max(out=m2col[:, bt : bt + 1], in_=p2, axis=X)

            # threshold shuffle partition->free via a DRAM bounce on the
            # GPSIMD (SWDGE) path, concurrent with the HWDGE chunk stream
            m2d = dpool.tile([BT, P], F32)
            nc.gpsimd.dma_start(out=m2d.rearrange("bt p -> p bt"), in_=m2col[:, :])
            thr_sb = spool.tile([NUM_CLASS, B], F32)
            nc.gpsimd.dma_start(
                out=thr_sb,
                in_=m2d.rearrange("bt p -> (bt p)")[None, :].broadcast_to(
                    [NUM_CLASS, B]
                ),
            )
            maskT = tpool.tile([NUM_CLASS, B], F32)
            nc.vector.tensor_tensor(
                out=maskT, in0=psT_sb, in1=thr_sb, op=mybir.AluOpType.is_ge
            )
            # ht: [mask*G rows 0:64][ones row 64]; ones row built early
            ht = tpool.tile([NUM_CLASS + 1, B], F16)
            nc.vector.tensor_scalar(
                out=ht[NUM_CLASS : NUM_CLASS + 1, :],
                in0=thr_sb[0:1, :],
                scalar1=0.0,
                scalar2=1.0,
                op0=mybir.AluOpType.mult,
                op1=mybir.AluOpType.add,
            )

            # ---- PSUM accumulators ----
            mps = [
                ppool.tile([P, OUT_L], F32, tag=f"main{bt}", name=f"main{bt}")
                for bt in range(BT)
            ]
            # G accumulates as two concurrent column-tiles (even k -> array
            # cols 0-63, odd k -> 64-127); issuing the two halves adjacently
            # makes them stream concurrently. The halves live in SEPARATE
            # PSUM banks so the even half's mask-multiply can run while the
            # odd half is still being written by the PE.
            gt0_ps = ppool.tile([2 * NUM_CLASS, B], F32, tag="gt0", name="gt0_ps")
            gt1_ps = ppool.tile([2 * NUM_CLASS, B], F32, tag="gt1", name="gt1_ps")

            def g_mm(k, xk):
                half = k % 2
                gt = gt0_ps if half == 0 else gt1_ps
                nc.tensor.matmul(
                    gt[half * NUM_CLASS : (half + 1) * NUM_CLASS, :],
                    lhsT=a_sb[:, k * NUM_CLASS : (k + 1) * NUM_CLASS],
                    rhs=xk,
                    start=(k == half),
                    stop=(k == KT - 2 + half),
                    tile_position=(0, half * NUM_CLASS),
                )

            def main_mms(k, xk, wk):
                for bt in range(BT):
                    nc.tensor.matmul(
                        mps[bt],
                        lhsT=xk[:, bt * P : (bt + 1) * P],
                        rhs=wk,
                        start=(k == 0),
                        stop=False,
                    )

            # ---- main streaming loop ----
            # mains for pairs of k, then the G pair adjacently so the two
            # column-tiles overlap on the array. For the final pair issue
            # the G matmuls before the last mains so the tail DVE chain
            # overlaps them.
            kslices = []
            for c, ck in enumerate(CHUNK_KS):
                t = chunk_tiles[c]
                woff = ck * B
                for kk in range(ck):
                    kslices.append(
                        (
                            t[:, kk * B : (kk + 1) * B],
                            t[:, woff + kk * OUT_L : woff + (kk + 1) * OUT_L],
                        )
                    )
            for k0 in range(0, KT, 2):
                (x0, w0), (x1, w1) = kslices[k0], kslices[k0 + 1]
                if k0 == KT - 2:
                    main_mms(k0, x0, w0)
                    g_mm(k0, x0)
                    g_mm(k0 + 1, x1)
                    main_mms(k0 + 1, x1, w1)
                else:
                    main_mms(k0, x0, w0)
                    main_mms(k0 + 1, x1, w1)
                    g_mm(k0, x0)
                    g_mm(k0 + 1, x1)

            # ---- tail ----
            # mask-multiply each G column-half straight out of PSUM (one
            # PSUM input per DVE op), merge into ht (f16), then the four
            # f16 tail matmuls back-to-back; copies alternate DVE/ACT and
            # the output DMAs alternate sync/gpsimd so nothing serializes
            # on a single engine.
            o_all = opool.tile([P, BT * OUT_L], F16)
            h0 = tpool.tile([NUM_CLASS, B], F32)
            h1 = tpool.tile([NUM_CLASS, B], F32)
            # h0 depends only on the even G half (stops at k=30, before the
            # final mains), so it overlaps the end of the stream
            nc.vector.tensor_tensor(
                out=h0, in0=gt0_ps[0:NUM_CLASS, :], in1=maskT,
                op=mybir.AluOpType.mult,
            )
            nc.vector.tensor_tensor(
                out=h1, in0=gt1_ps[NUM_CLASS : 2 * NUM_CLASS, :], in1=maskT,
                op=mybir.AluOpType.mult,
            )
            nc.vector.tensor_tensor(
                out=ht[0:NUM_CLASS, :], in0=h0, in1=h1, op=mybir.AluOpType.add
            )
            for bt in range(BT):
                sl = slice(bt * P, (bt + 1) * P)
                nc.tensor.matmul(
                    mps[bt],
                    lhsT=ht[:, sl],
                    rhs=bS_sb,
                    start=False,
                    stop=True,
                )
            for bt in range(BT):
                sl = slice(bt * P, (bt + 1) * P)
                osl = o_all[:, bt * OUT_L : (bt + 1) * OUT_L]
                if bt % 2 == 0:
                    nc.vector.tensor_copy(out=osl, in_=mps[bt])
                    nc.sync.dma_start(out=out[sl, :], in_=osl)
                else:
                    nc.scalar.copy(out=osl, in_=mps[bt])
                    nc.scalar.dma_start(out=out[sl, :], in_=osl)

    nc.finalize()
    _cache[key] = nc
    return nc


def _pack_inputs(x, pseudo_index, weight, bias, lora_A, lora_B):
    """Build the interleaved per-core xw chunk buffers + replicated small
    inputs (lora_A chunks, pseudo_index layouts, scaled lora_B/bias)."""
    xT = np.ascontiguousarray(x.T).astype(np.float16)   # [IN, B]
    aT = lora_A[:NUM_CLASS].T.astype(np.float16)        # [IN, 64]

    # av: [128, AVW]: [p, k*64+c] = aT[k*128+p, c], then bS (per-core)
    av_base = np.zeros((P, AVW), dtype=np.float16)
    av_base[:, :AW] = (
        aT.reshape(KT, P, NUM_CLASS).transpose(1, 0, 2).reshape(P, KT * NUM_CLASS)
    )

    pp_base = np.zeros((P, PPW), dtype=np.float32)
    pp_base[:, PSOFF : PSOFF + BT * NUM_CLASS] = (
        pseudo_index.reshape(BT, P, NUM_CLASS)
        .transpose(1, 0, 2)
        .reshape(P, BT * NUM_CLASS)
    )
    pp_base[:NUM_CLASS, PTOFF : PTOFF + B] = pseudo_index.T

    # per-chunk x blocks: for chunk c with k-tiles [k0, k0+ck):
    #   [p, kk*B + b] = xT[(k0+kk)*P + p, b]
    x3 = xT.reshape(KT, P, B)  # [k, p, b]

    in_maps = []
    for i in range(NCORES):
        o0 = i * OUT_L
        wTi = weight[o0 : o0 + OUT_L].T.astype(np.float16)  # [IN, OUT_L]
        w3 = wTi.reshape(KT, P, OUT_L)
        xwi = np.zeros((NCHUNK, P, XW_WIDTH), dtype=np.float16)
        k0 = 0
        for c, ck in enumerate(CHUNK_KS):
            xwi[c, :, : ck * B] = (
                x3[k0 : k0 + ck].transpose(1, 0, 2).reshape(P, ck * B)
            )
            xwi[c, :, ck * B : ck * (B + OUT_L)] = (
                w3[k0 : k0 + ck].transpose(1, 0, 2).reshape(P, ck * OUT_L)
            )
            k0 += ck
        avi = av_base.copy()
        avi[:NUM_CLASS, AW:AVW] = 16.0 * lora_B[o0 : o0 + OUT_L, :NUM_CLASS].T
        avi[NUM_CLASS, AW:AVW] = 2.0 * bias[o0 : o0 + OUT_L]
        in_maps.append({"xw": xwi, "av": avi, "pp": pp_base})
    return in_maps


def kernel(x, pseudo_index, weight, bias, lora_A, lora_B):
    global last_results
    x = np.ascontiguousarray(np.asarray(x, dtype=np.float32))
    pseudo_index = np.ascontiguousarray(np.asarray(pseudo_index, dtype=np.float32))
    weight = np.asarray(weight, dtype=np.float32)
    bias = np.asarray(bias, dtype=np.float32)
    lora_A = np.asarray(lora_A, dtype=np.float32)
    lora_B = np.asarray(lora_B, dtype=np.float32)

    nc = _build()
    in_maps = _pack_inputs(x, pseudo_index, weight, bias, lora_A, lora_B)
    res = run_bass_kernel_spmd(nc, in_maps, list(range(NCORES)))
    last_results = res
    return np.hstack(
        [res.results[i]["out"].astype(np.float32) for i in range(NCORES)]
    )


# revision 24
# speedup vs baseline: 1.0908x; 1.0077x over previous
"""CALoraLinear kernel for 8 TRN2 NeuronCores (Bass/Tile, SPMD).

Math (derived from the reference):
  orig = x @ W.T + bias
  top2 classes c1,c2 per row from pseudo_index[b, :64]
  g_j = <lora_A[c_j], x[b]>          (only rows 0..63 of lora_A are reachable)
  lora_out[b,o] = 16 * sum_c mask[b,c] * G[b,c] * lora_B[o,c]
  out = orig + lora_out + bias       (bias added twice)

Sharding: column-shard W across the 8 cores (each core owns 512 output
columns, full batch); x / lora_A / pseudo_index replicated. Host
concatenates the per-core [512, 512] blocks along the output axis.

v2 design (PE-bound; DMA measured at ~420 GB/s/core sustained):
  - f16 streams for x and W (PE streams 1 col/cycle; fp8 would need
    error compensation that costs more than it saves).
  - lora_A + the small pack (pseudo_index layouts, scaled lora_B, bias)
    are loaded once up front; only x|W stream per chunk. First chunks
    are small (2 k-tiles) to cut time-to-first-matmul, then 4 k-tiles.
  - A dummy-matmul warmup train runs during the DMA preamble so the PE
    HAM clock gate is already at 8/8 when real data lands.
  - G matmuls are issued as ADJACENT column-tile pairs (even k -> array
    cols 0-63, odd k -> 64-127) so each pair streams concurrently:
    512 cycles per 2 k-tiles instead of 1024.
  - Threshold/mask pipeline (top-2 over 64 classes) runs on DVE/SWDGE
    entirely under the main stream.
  - Tail is pipelined per batch-tile: gts+mask -> f32r tail matmul into
    the main PSUM bank -> PSUM->SBUF copy (alternating Vector/Scalar
    engines) -> f16 output DMA (host upcasts to f32).
"""

import os
import sys

for _p in ("/opt/trn_rl_repo",):
    if _p not in sys.path:
        sys.path.insert(0, _p)

import numpy as np

import concourse.bass as bass
import concourse.bacc as bacc
import concourse.mybir as mybir
from concourse.tile import TileContext, add_dep_helper
from concourse.bass_utils import run_bass_kernel_spmd


def _ensure_ntff_hook_module():
    """run_bass_kernel_spmd(trace=True) imports antenv.axon_hooks, which the
    agent image's antenv package lacks. Provide it (and register the real
    ctypes NTFF hook when available) so a tracing caller doesn't crash."""
    import types

    try:
        import antenv
    except ImportError:
        return
    if getattr(antenv, "axon_hooks", None) is not None:
        return
    mod = types.ModuleType("antenv.axon_hooks")
    state = {"hook": None}
    mod.set_axon_ntff_profile_hook = lambda h: state.__setitem__("hook", h)
    mod.get_axon_ntff_profile_hook = lambda: state["hook"]
    sys.modules["antenv.axon_hooks"] = mod
    antenv.axon_hooks = mod
    try:
        from trn_agent_boot.trn_boot import _ntff_profile_via_ctypes

        mod.set_axon_ntff_profile_hook(
            _ntff_profile_via_ctypes("/opt/axon/libaxon_pjrt.so")
        )
    except Exception:
        pass


_ensure_ntff_hook_module()

B, IN, OUT = 512, 4096, 4096
NUM_CLASS, RANK = 64, 8
NCORES = 8
OUT_L = OUT // NCORES  # 512
P = 128
KT = IN // P           # 32 k-tiles
BT = B // P            # 4 batch tiles

# chunk schedule: k-tiles per streamed chunk (first ones small to cut
# time-to-first-matmul and ride out the DMA ramp, last ones small to
# shrink the end-of-stream residual work)
CHUNK_KS = [1, 1, 2, 2, 2, 2, 4, 4, 4, 4, 4, 2]
assert sum(CHUNK_KS) == KT
NCHUNK = len(CHUNK_KS)

# column layout inside one chunk of ck k-tiles: [x: ck*B][w: ck*OUT_L]
def _chunk_width(ck):
    return ck * (B + OUT_L)

XW_WIDTH = max(_chunk_width(ck) for ck in CHUNK_KS)  # 4096

# pp layout: [ps: BT*64][psT: B]
PSOFF = 0
PTOFF = BT * NUM_CLASS          # 256
PPW = PTOFF + B                 # 768

# av layout (f16): [lora_A chunks: KT*64][bS: OUT_L (rows 0:65)]
AW = KT * NUM_CLASS             # 2048
AVW = AW + OUT_L                # 2560
# av is DMA'd in two pieces: head (lora_A for k<16, needed early for the
# first G matmuls) and tail (rest of lora_A + bS, needed later)
AHW = (KT // 2) * NUM_CLASS     # 1024

F32 = mybir.dt.float32
F32R = mybir.dt.float32r
F16 = mybir.dt.float16
X = mybir.AxisListType.X

NWARM = 96  # dummy warmup matmuls (N=64 each, ~55ns cold) ~ 5.4us; must
            # bridge the PE from kernel start to first-chunk arrival so
            # the HAM clock-gate stays at 8/8 for the real matmuls

_cache = {}
# test.py reads this after a traced run for HW exec time
last_results = None


def _build():
    key = "nc_v2"
    if key in _cache:
        return _cache[key]
    nc = bacc.Bacc(
        bass.get_trn_type() or "TRN2",
        target_bir_lowering=False,
        debug=False,
        num_devices=NCORES,
    )

    xw = nc.dram_tensor("xw", [NCHUNK, P, XW_WIDTH], F16, kind="ExternalInput")
    av = nc.dram_tensor("av", [P, AVW], F16, kind="ExternalInput")
    pp = nc.dram_tensor("pp", [P, PPW], F32R, kind="ExternalInput")
    out = nc.dram_tensor("out", [B, OUT_L], F16, kind="ExternalOutput")

    with TileContext(nc) as tc:
        with (
            tc.tile_pool(name="xwp", bufs=1) as xwpool,
            tc.tile_pool(name="sml", bufs=1) as spool,
            tc.tile_pool(name="tl", bufs=1) as tpool,
            tc.tile_pool(name="op", bufs=1) as opool,
            tc.tile_pool(name="dr", bufs=1, space="DRAM") as dpool,
            tc.tile_pool(name="ps", bufs=1, space="PSUM") as ppool,
        ):
            # ---- input DMAs, issued first. Each HWDGE queue serves its
            # DMAs FIFO at ~half the aggregate rate, so each queue's order
            # must match PE consumption order: sync = even chunks (+ the
            # output writes later), scalar = lora_A then odd chunks, and
            # pp rides the SWDGE (gpsimd) queue — it only feeds the
            # threshold pipeline, which is off the critical path. ----
            chunk_tiles = []
            # c0 leads the sync queue, c1 the scalar queue; pp rides the
            # SWDGE (gpsimd) queue which only feeds the off-critical-path
            # threshold pipeline.
            c0 = xwpool.tile([P, _chunk_width(CHUNK_KS[0])], F16, tag="xwc0",
                             name="xwc0")
            nc.sync.dma_start(out=c0, in_=xw[0, :, : _chunk_width(CHUNK_KS[0])])
            chunk_tiles.append(c0)

            c1 = xwpool.tile([P, _chunk_width(CHUNK_KS[1])], F16, tag="xwc1",
                             name="xwc1")
            nc.sync.dma_start(out=c1, in_=xw[1, :, : _chunk_width(CHUNK_KS[1])])
            chunk_tiles.append(c1)

            av_sb = spool.tile([P, AVW], F16)
            nc.scalar.dma_start(out=av_sb[:, :AHW], in_=av[:, :AHW])
            pp_sb = spool.tile([P, PPW], F32R)
            nc.gpsimd.dma_start(out=pp_sb, in_=pp[:, :])

            a_sb = av_sb[:, :AW]
            bS_sb = av_sb[: NUM_CLASS + 1, AW:AVW]

            # sync gets the early ladder (c2, then alternating), scalar the
            # rest; each queue's FIFO order matches PE consumption order
            SYNC_CHUNKS = {2, 4, 6, 8, 10}
            av_tail_at = NCHUNK // 2  # dispatch position for av tail piece
            for c in range(2, NCHUNK):
                w = _chunk_width(CHUNK_KS[c])
                t = xwpool.tile([P, w], F16, tag=f"xwc{c}", name=f"xwc{c}")
                eng = nc.sync if c in SYNC_CHUNKS else nc.scalar
                if c == av_tail_at:
                    nc.scalar.dma_start(out=av_sb[:, AHW:], in_=av[:, AHW:])
                dma = eng.dma_start(out=t, in_=xw[c, :, :w])
                chunk_tiles.append(t)

            ps_sb = pp_sb[:, PSOFF : PSOFF + BT * NUM_CLASS].bitcast(F32)
            psT_sb = pp_sb[:NUM_CLASS, PTOFF : PTOFF + B].bitcast(F32)

            # ---- PE warmup train: keep the HAM clock-gate busy while the
            # first chunks stream in, so real matmuls start at 2.4 GHz ----
            warm_sb = spool.tile([P, NUM_CLASS], F16)
            nc.vector.memset(warm_sb, 0.0)
            warm_ps = ppool.tile([NUM_CLASS, NUM_CLASS], F32, tag="warm",
                                 name="warm_ps")
            for _ in range(NWARM):
                nc.tensor.matmul(
                    warm_ps, lhsT=warm_sb, rhs=warm_sb[:, :NUM_CLASS],
                    start=True, stop=True,
                )

            # ---- top-2 threshold per batch row (DVE, under the stream) ----
            m2col = spool.tile([P, BT], F32)
            for bt in range(BT):
                pt = ps_sb[:, bt * NUM_CLASS : (bt + 1) * NUM_CLASS]
                m1 = spool.tile([P, 1], F32, tag=f"m1_{bt}")
                nc.vector.reduce_max(out=m1, in_=pt, axis=X)
                negmask = spool.tile([P, NUM_CLASS], F32, tag=f"nm_{bt}")
                # (pt >= m1) * -1e30  -> additive mask that kills the max
                nc.vector.tensor_scalar(
                    out=negmask,
                    in0=pt,
                    scalar1=m1,
                    scalar2=-1.0e30,
                    op0=mybir.AluOpType.is_ge,
                    op1=mybir.AluOpType.mult,
                )
                p2 = spool.tile([P, NUM_CLASS], F32, tag=f"p2_{bt}")
                nc.vector.tensor_tensor(
                    out=p2, in0=pt, in1=negmask, op=mybir.AluOpType.add
                )
                nc.vector.reduce_max(out=m2col[:, bt : bt + 1], in_=p2, axis=X)

            # threshold shuffle partition->free via a DRAM bounce on the
            # GPSIMD (SWDGE) path, concurrent with the HWDGE chunk stream
            m2d = dpool.tile([BT, P], F32)
            nc.gpsimd.dma_start(out=m2d.rearrange("bt p -> p bt"), in_=m2col[:, :])
            thr_sb = spool.tile([NUM_CLASS, B], F32)
            nc.gpsimd.dma_start(
                out=thr_sb,
                in_=m2d.rearrange("bt p -> (bt p)")[None, :].broadcast_to(
                    [NUM_CLASS, B]
                ),
            )
            maskT = tpool.tile([NUM_CLASS, B], F32)
            nc.vector.tensor_tensor(
                out=maskT, in0=psT_sb, in1=thr_sb, op=mybir.AluOpType.is_ge
            )
            # ht0: [mask*G_even rows 0:64][ones row 64]; ht1: mask*G_odd.
            # The ones row (built early) carries the doubled bias via the
            # last row of bS; splitting the tail matmul per G-half removes
            # the DVE merge from the end-of-kernel critical path.
            ht0 = tpool.tile([NUM_CLASS + 1, B], F16)
            ht1 = tpool.tile([NUM_CLASS, B], F16)
            nc.vector.tensor_scalar(
                out=ht0[NUM_CLASS : NUM_CLASS + 1, :],
                in0=thr_sb[0:1, :],
                scalar1=0.0,
                scalar2=1.0,
                op0=mybir.AluOpType.mult,
                op1=mybir.AluOpType.add,
            )

            # ---- PSUM accumulators ----
            mps = [
                ppool.tile([P, OUT_L], F32, tag=f"main{bt}", name=f"main{bt}")
                for bt in range(BT)
            ]
            # G accumulates as two concurrent column-tiles (even k -> array
            # cols 0-63, odd k -> 64-127); issuing the two halves adjacently
            # makes them stream concurrently. The halves live in SEPARATE
            # PSUM banks so the even half's mask-multiply can run while the
            # odd half is still being written by the PE.
            gt0_ps = ppool.tile([2 * NUM_CLASS, B], F32, tag="gt0", name="gt0_ps")
            gt1_ps = ppool.tile([2 * NUM_CLASS, B], F32, tag="gt1", name="gt1_ps")

            def g_mm(k, xk):
                half = k % 2
                gt = gt0_ps if half == 0 else gt1_ps
                nc.tensor.matmul(
                    gt[half * NUM_CLASS : (half + 1) * NUM_CLASS, :],
                    lhsT=a_sb[:, k * NUM_CLASS : (k + 1) * NUM_CLASS],
                    rhs=xk,
                    start=(k == half),
                    stop=(k == KT - 2 + half),
                    tile_position=(0, half * NUM_CLASS),
                )

            def main_mms(k, xk, wk):
                for bt in range(BT):
                    nc.tensor.matmul(
                        mps[bt],
                        lhsT=xk[:, bt * P : (bt + 1) * P],
                        rhs=wk,
                        start=(k == 0),
                        stop=False,
                    )

            # ---- main streaming loop ----
            # mains for pairs of k, then the G pair adjacently so the two
            # column-tiles overlap on the array. For the final pair issue
            # the G matmuls before the last mains so the tail DVE chain
            # overlaps them.
            kslices = []
            for c, ck in enumerate(CHUNK_KS):
                t = chunk_tiles[c]
                woff = ck * B
                for kk in range(ck):
                    kslices.append(
                        (
                            t[:, kk * B : (kk + 1) * B],
                            t[:, woff + kk * OUT_L : woff + (kk + 1) * OUT_L],
                        )
                    )
            for k0 in range(0, KT, 2):
                (x0, w0), (x1, w1) = kslices[k0], kslices[k0 + 1]
                if k0 == KT - 2:
                    main_mms(k0, x0, w0)
                    g_mm(k0, x0)
                    g_mm(k0 + 1, x1)
                    main_mms(k0 + 1, x1, w1)
                else:
                    main_mms(k0, x0, w0)
                    main_mms(k0 + 1, x1, w1)
                    g_mm(k0, x0)
                    g_mm(k0 + 1, x1)

            # ---- tail ----
            # mask-multiply each G column-half straight out of PSUM (one
            # PSUM input per DVE op), merge into ht (f16), then the four
            # f16 tail matmuls back-to-back; copies alternate DVE/ACT and
            # the output DMAs alternate sync/gpsimd so nothing serializes
            # on a single engine.
            o_all = opool.tile([P, BT * OUT_L], F16)
            # ht0 depends only on the even G half (stops at k=30, before
            # the final mains) so both mask-multiplies overlap the end of
            # the stream; the tail is then pure back-to-back matmuls
            nc.vector.tensor_tensor(
                out=ht0[0:NUM_CLASS, :], in0=gt0_ps[0:NUM_CLASS, :],
                in1=maskT, op=mybir.AluOpType.mult,
            )
            nc.vector.tensor_tensor(
                out=ht1, in0=gt1_ps[NUM_CLASS : 2 * NUM_CLASS, :],
                in1=maskT, op=mybir.AluOpType.mult,
            )
            for bt in range(BT):
                sl = slice(bt * P, (bt + 1) * P)
                nc.tensor.matmul(
                    mps[bt],
                    lhsT=ht0[:, sl],
                    rhs=bS_sb,
                    start=False,
                    stop=False,
                )
                nc.tensor.matmul(
                    mps[bt],
                    lhsT=ht1[:, sl],
                    rhs=bS_sb[0:NUM_CLASS, :],
                    start=False,
                    stop=True,
                )
            for bt in range(BT):
                sl = slice(bt * P, (bt + 1) * P)
                osl = o_all[:, bt * OUT_L : (bt + 1) * OUT_L]
                if bt % 2 == 0:
                    nc.vector.tensor_copy(out=osl, in_=mps[bt])
                    nc.sync.dma_start(out=out[sl, :], in_=osl)
                else:
                    nc.scalar.copy(out=osl, in_=mps[bt])
                    nc.scalar.dma_start(out=out[sl, :], in_=osl)

    nc.finalize()
    _cache[key] = nc
    return nc


def _pack_inputs(x, pseudo_index, weight, bias, lora_A, lora_B):
    """Build the interleaved per-core xw chunk buffers + replicated small
    inputs (lora_A chunks, pseudo_index layouts, scaled lora_B/bias)."""
    xT = np.ascontiguousarray(x.T).astype(np.float16)   # [IN, B]
    aT = lora_A[:NUM_CLASS].T.astype(np.float16)        # [IN, 64]

    # av: [128, AVW]: [p, k*64+c] = aT[k*128+p, c], then bS (per-core)
    av_base = np.zeros((P, AVW), dtype=np.float16)
    av_base[:, :AW] = (
        aT.reshape(KT, P, NUM_CLASS).transpose(1, 0, 2).reshape(P, KT * NUM_CLASS)
    )

    pp_base = np.zeros((P, PPW), dtype=np.float32)
    pp_base[:, PSOFF : PSOFF + BT * NUM_CLASS] = (
        pseudo_index.reshape(BT, P, NUM_CLASS)
        .transpose(1, 0, 2)
        .reshape(P, BT * NUM_CLASS)
    )
    pp_base[:NUM_CLASS, PTOFF : PTOFF + B] = pseudo_index.T

    # per-chunk x blocks: for chunk c with k-tiles [k0, k0+ck):
    #   [p, kk*B + b] = xT[(k0+kk)*P + p, b]
    x3 = xT.reshape(KT, P, B)  # [k, p, b]

    in_maps = []
    for i in range(NCORES):
        o0 = i * OUT_L
        wTi = weight[o0 : o0 + OUT_L].T.astype(np.float16)  # [IN, OUT_L]
        w3 = wTi.reshape(KT, P, OUT_L)
        xwi = np.zeros((NCHUNK, P, XW_WIDTH), dtype=np.float16)
        k0 = 0
        for c, ck in enumerate(CHUNK_KS):
            xwi[c, :, : ck * B] = (
                x3[k0 : k0 + ck].transpose(1, 0, 2).reshape(P, ck * B)
            )
            xwi[c, :, ck * B : ck * (B + OUT_L)] = (
                w3[k0 : k0 + ck].transpose(1, 0, 2).reshape(P, ck * OUT_L)
            )
            k0 += ck
        avi = av_base.copy()
        avi[:NUM_CLASS, AW:AVW] = 16.0 * lora_B[o0 : o0 + OUT_L, :NUM_CLASS].T
        avi[NUM_CLASS, AW:AVW] = 2.0 * bias[o0 : o0 + OUT_L]
        in_maps.append({"xw": xwi, "av": avi, "pp": pp_base})
    return in_maps


def kernel(x, pseudo_index, weight, bias, lora_A, lora_B):
    global last_results
    x = np.ascontiguousarray(np.asarray(x, dtype=np.float32))
    pseudo_index = np.ascontiguousarray(np.asarray(pseudo_index, dtype=np.float32))
    weight = np.asarray(weight, dtype=np.float32)
    bias = np.asarray(bias, dtype=np.float32)
    lora_A = np.asarray(lora_A, dtype=np.float32)
    lora_B = np.asarray(lora_B, dtype=np.float32)

    nc = _build()
    in_maps = _pack_inputs(x, pseudo_index, weight, bias, lora_A, lora_B)
    res = run_bass_kernel_spmd(nc, in_maps, list(range(NCORES)))
    last_results = res
    return np.hstack(
        [res.results[i]["out"].astype(np.float32) for i in range(NCORES)]
    )


# revision 25
# speedup vs baseline: 1.1326x; 1.0384x over previous
"""CALoraLinear kernel for 8 TRN2 NeuronCores (Bass/Tile, SPMD).

Math (derived from the reference):
  orig = x @ W.T + bias
  top2 classes c1,c2 per row from pseudo_index[b, :64]
  g_j = <lora_A[c_j], x[b]>          (only rows 0..63 of lora_A are reachable)
  lora_out[b,o] = 16 * sum_c mask[b,c] * G[b,c] * lora_B[o,c]
  out = orig + lora_out + bias       (bias added twice)

Sharding: column-shard W across the 8 cores (each core owns 512 output
columns, full batch); x / lora_A / pseudo_index replicated. Host
concatenates the per-core [512, 512] blocks along the output axis.

v2 design (PE-bound; DMA measured at ~420 GB/s/core sustained):
  - f16 streams for x and W (PE streams 1 col/cycle; fp8 would need
    error compensation that costs more than it saves).
  - lora_A + the small pack (pseudo_index layouts, scaled lora_B, bias)
    are loaded once up front; only x|W stream per chunk. First chunks
    are small (2 k-tiles) to cut time-to-first-matmul, then 4 k-tiles.
  - A dummy-matmul warmup train runs during the DMA preamble so the PE
    HAM clock gate is already at 8/8 when real data lands.
  - G matmuls are issued as ADJACENT column-tile pairs (even k -> array
    cols 0-63, odd k -> 64-127) so each pair streams concurrently:
    512 cycles per 2 k-tiles instead of 1024.
  - Threshold/mask pipeline (top-2 over 64 classes) runs on DVE/SWDGE
    entirely under the main stream.
  - Tail is pipelined per batch-tile: gts+mask -> f32r tail matmul into
    the main PSUM bank -> PSUM->SBUF copy (alternating Vector/Scalar
    engines) -> f16 output DMA (host upcasts to f32).
"""

import os
import sys

for _p in ("/opt/trn_rl_repo",):
    if _p not in sys.path:
        sys.path.insert(0, _p)

import numpy as np

import concourse.bass as bass
import concourse.bacc as bacc
import concourse.mybir as mybir
from concourse.tile import TileContext, add_dep_helper
from concourse.bass_utils import run_bass_kernel_spmd


def _ensure_ntff_hook_module():
    """run_bass_kernel_spmd(trace=True) imports antenv.axon_hooks, which the
    agent image's antenv package lacks. Provide it (and register the real
    ctypes NTFF hook when available) so a tracing caller doesn't crash."""
    import types

    try:
        import antenv
    except ImportError:
        return
    if getattr(antenv, "axon_hooks", None) is not None:
        return
    mod = types.ModuleType("antenv.axon_hooks")
    state = {"hook": None}
    mod.set_axon_ntff_profile_hook = lambda h: state.__setitem__("hook", h)
    mod.get_axon_ntff_profile_hook = lambda: state["hook"]
    sys.modules["antenv.axon_hooks"] = mod
    antenv.axon_hooks = mod
    try:
        from trn_agent_boot.trn_boot import _ntff_profile_via_ctypes

        mod.set_axon_ntff_profile_hook(
            _ntff_profile_via_ctypes("/opt/axon/libaxon_pjrt.so")
        )
    except Exception:
        pass


_ensure_ntff_hook_module()

B, IN, OUT = 512, 4096, 4096
NUM_CLASS, RANK = 64, 8
NCORES = 8
OUT_L = OUT // NCORES  # 512
P = 128
KT = IN // P           # 32 k-tiles
BT = B // P            # 4 batch tiles

# chunk schedule: k-tiles per streamed chunk (first ones small to cut
# time-to-first-matmul and ride out the DMA ramp, last ones small to
# shrink the end-of-stream residual work)
CHUNK_KS = [1, 1, 2, 2, 2, 2, 4, 4, 4, 4, 4, 2]
assert sum(CHUNK_KS) == KT
NCHUNK = len(CHUNK_KS)

# column layout inside one chunk of ck k-tiles: [x: ck*B][w: ck*OUT_L]
def _chunk_width(ck):
    return ck * (B + OUT_L)

XW_WIDTH = max(_chunk_width(ck) for ck in CHUNK_KS)  # 4096

# pp layout: [ps: BT*64][psT: B]
PSOFF = 0
PTOFF = BT * NUM_CLASS          # 256
PPW = PTOFF + B                 # 768

# av layout (f16): [lora_A chunks: KT*64][bS: OUT_L (rows 0:65)]
AW = KT * NUM_CLASS             # 2048
AVW = AW + OUT_L                # 2560
# av is DMA'd in two pieces: head (lora_A for k<16, needed early for the
# first G matmuls) and tail (rest of lora_A + bS, needed later)
AHW = (KT // 2) * NUM_CLASS     # 1024

F32 = mybir.dt.float32
F32R = mybir.dt.float32r
F16 = mybir.dt.float16
X = mybir.AxisListType.X

NWARM = 96  # dummy warmup matmuls (N=64 each, ~55ns cold) ~ 5.4us; must
            # bridge the PE from kernel start to first-chunk arrival so
            # the HAM clock-gate stays at 8/8 for the real matmuls

_cache = {}
# test.py reads this after a traced run for HW exec time
last_results = None


def _build():
    key = "nc_v2"
    if key in _cache:
        return _cache[key]
    nc = bacc.Bacc(
        bass.get_trn_type() or "TRN2",
        target_bir_lowering=False,
        debug=False,
        num_devices=NCORES,
    )

    xw = nc.dram_tensor("xw", [NCHUNK, P, XW_WIDTH], F16, kind="ExternalInput")
    av = nc.dram_tensor("av", [P, AVW], F16, kind="ExternalInput")
    pp = nc.dram_tensor("pp", [P, PPW], F32R, kind="ExternalInput")
    out = nc.dram_tensor("out", [B, OUT_L], F16, kind="ExternalOutput")

    with TileContext(nc) as tc:
        with (
            tc.tile_pool(name="xwp", bufs=1) as xwpool,
            tc.tile_pool(name="sml", bufs=1) as spool,
            tc.tile_pool(name="tl", bufs=1) as tpool,
            tc.tile_pool(name="op", bufs=1) as opool,
            tc.tile_pool(name="dr", bufs=1, space="DRAM") as dpool,
            tc.tile_pool(name="ps", bufs=1, space="PSUM") as ppool,
        ):
            # ---- input DMAs, issued first. Each HWDGE queue serves its
            # DMAs FIFO at ~half the aggregate rate, so each queue's order
            # must match PE consumption order: sync = even chunks (+ the
            # output writes later), scalar = lora_A then odd chunks, and
            # pp rides the SWDGE (gpsimd) queue — it only feeds the
            # threshold pipeline, which is off the critical path. ----
            chunk_tiles = []
            # c0 leads the sync queue, c1 the scalar queue; pp rides the
            # SWDGE (gpsimd) queue which only feeds the off-critical-path
            # threshold pipeline.
            c0 = xwpool.tile([P, _chunk_width(CHUNK_KS[0])], F16, tag="xwc0",
                             name="xwc0")
            nc.sync.dma_start(out=c0, in_=xw[0, :, : _chunk_width(CHUNK_KS[0])])
            chunk_tiles.append(c0)

            c1 = xwpool.tile([P, _chunk_width(CHUNK_KS[1])], F16, tag="xwc1",
                             name="xwc1")
            nc.sync.dma_start(out=c1, in_=xw[1, :, : _chunk_width(CHUNK_KS[1])])
            chunk_tiles.append(c1)

            av_sb = spool.tile([P, AVW], F16)
            nc.scalar.dma_start(out=av_sb[:, :AHW], in_=av[:, :AHW])
            pp_sb = spool.tile([P, PPW], F32R)
            nc.gpsimd.dma_start(out=pp_sb, in_=pp[:, :])

            a_sb = av_sb[:, :AW]
            bS_sb = av_sb[: NUM_CLASS + 1, AW:AVW]

            # sync gets the early ladder (c2, then alternating), scalar the
            # rest; each queue's FIFO order matches PE consumption order
            SYNC_CHUNKS = {2, 4, 6, 8, 10}
            av_tail_at = NCHUNK // 2  # dispatch position for av tail piece
            for c in range(2, NCHUNK):
                w = _chunk_width(CHUNK_KS[c])
                t = xwpool.tile([P, w], F16, tag=f"xwc{c}", name=f"xwc{c}")
                eng = nc.sync if c in SYNC_CHUNKS else nc.scalar
                if c == av_tail_at:
                    nc.scalar.dma_start(out=av_sb[:, AHW:], in_=av[:, AHW:])
                dma = eng.dma_start(out=t, in_=xw[c, :, :w])
                chunk_tiles.append(t)

            ps_sb = pp_sb[:, PSOFF : PSOFF + BT * NUM_CLASS].bitcast(F32)
            psT_sb = pp_sb[:NUM_CLASS, PTOFF : PTOFF + B].bitcast(F32)

            # ---- PE warmup train: keep the HAM clock-gate busy while the
            # first chunks stream in, so real matmuls start at 2.4 GHz ----
            warm_sb = spool.tile([P, NUM_CLASS], F16)
            nc.vector.memset(warm_sb, 0.0)
            warm_ps = ppool.tile([NUM_CLASS, NUM_CLASS], F32, tag="warm",
                                 name="warm_ps")
            for _ in range(NWARM):
                nc.tensor.matmul(
                    warm_ps, lhsT=warm_sb, rhs=warm_sb[:, :NUM_CLASS],
                    start=True, stop=True,
                )

            # ---- top-2 threshold per batch row (DVE, under the stream) ----
            m2col = spool.tile([P, BT], F32)
            for bt in range(BT):
                pt = ps_sb[:, bt * NUM_CLASS : (bt + 1) * NUM_CLASS]
                m1 = spool.tile([P, 1], F32, tag=f"m1_{bt}")
                nc.vector.reduce_max(out=m1, in_=pt, axis=X)
                negmask = spool.tile([P, NUM_CLASS], F32, tag=f"nm_{bt}")
                # (pt >= m1) * -1e30  -> additive mask that kills the max
                nc.vector.tensor_scalar(
                    out=negmask,
                    in0=pt,
                    scalar1=m1,
                    scalar2=-1.0e30,
                    op0=mybir.AluOpType.is_ge,
                    op1=mybir.AluOpType.mult,
                )
                p2 = spool.tile([P, NUM_CLASS], F32, tag=f"p2_{bt}")
                nc.vector.tensor_tensor(
                    out=p2, in0=pt, in1=negmask, op=mybir.AluOpType.add
                )
                nc.vector.reduce_max(out=m2col[:, bt : bt + 1], in_=p2, axis=X)

            # threshold shuffle partition->free via a DRAM bounce on the
            # GPSIMD (SWDGE) path, concurrent with the HWDGE chunk stream
            m2d = dpool.tile([BT, P], F32)
            nc.gpsimd.dma_start(out=m2d.rearrange("bt p -> p bt"), in_=m2col[:, :])
            thr_sb = spool.tile([NUM_CLASS, B], F32)
            nc.gpsimd.dma_start(
                out=thr_sb,
                in_=m2d.rearrange("bt p -> (bt p)")[None, :].broadcast_to(
                    [NUM_CLASS, B]
                ),
            )
            maskT = tpool.tile([NUM_CLASS, B], F32)
            nc.vector.tensor_tensor(
                out=maskT, in0=psT_sb, in1=thr_sb, op=mybir.AluOpType.is_ge
            )
            # ht0: [mask*G_even rows 0:64][ones row 64]; ht1: mask*G_odd.
            # The ones row (built early) carries the doubled bias via the
            # last row of bS; splitting the tail matmul per G-half removes
            # the DVE merge from the end-of-kernel critical path.
            ht0 = tpool.tile([NUM_CLASS + 1, B], F16)
            ht1 = tpool.tile([NUM_CLASS, B], F16)
            nc.vector.tensor_scalar(
                out=ht0[NUM_CLASS : NUM_CLASS + 1, :],
                in0=thr_sb[0:1, :],
                scalar1=0.0,
                scalar2=1.0,
                op0=mybir.AluOpType.mult,
                op1=mybir.AluOpType.add,
            )

            # ---- PSUM accumulators ----
            mps = [
                ppool.tile([P, OUT_L], F32, tag=f"main{bt}", name=f"main{bt}")
                for bt in range(BT)
            ]
            # G accumulates as two concurrent column-tiles (even k -> array
            # cols 0-63, odd k -> 64-127); issuing the two halves adjacently
            # makes them stream concurrently. The halves live in SEPARATE
            # PSUM banks so the even half's mask-multiply can run while the
            # odd half is still being written by the PE.
            gt0_ps = ppool.tile([2 * NUM_CLASS, B], F32, tag="gt0", name="gt0_ps")
            gt1_ps = ppool.tile([2 * NUM_CLASS, B], F32, tag="gt1", name="gt1_ps")

            def g_mm(k, xk):
                half = k % 2
                gt = gt0_ps if half == 0 else gt1_ps
                nc.tensor.matmul(
                    gt[half * NUM_CLASS : (half + 1) * NUM_CLASS, :],
                    lhsT=a_sb[:, k * NUM_CLASS : (k + 1) * NUM_CLASS],
                    rhs=xk,
                    start=(k == half),
                    stop=(k == KT - 2 + half),
                    tile_position=(0, half * NUM_CLASS),
                )

            def main_mms(k, xk, wk):
                for bt in range(BT):
                    nc.tensor.matmul(
                        mps[bt],
                        lhsT=xk[:, bt * P : (bt + 1) * P],
                        rhs=wk,
                        start=(k == 0),
                        stop=False,
                    )

            # ---- main streaming loop ----
            # mains for pairs of k, then the G pair adjacently so the two
            # column-tiles overlap on the array. For the final pair issue
            # the G matmuls before the last mains so the tail DVE chain
            # overlaps them.
            kslices = []
            for c, ck in enumerate(CHUNK_KS):
                t = chunk_tiles[c]
                woff = ck * B
                for kk in range(ck):
                    kslices.append(
                        (
                            t[:, kk * B : (kk + 1) * B],
                            t[:, woff + kk * OUT_L : woff + (kk + 1) * OUT_L],
                        )
                    )
            for k0 in range(0, KT, 2):
                (x0, w0), (x1, w1) = kslices[k0], kslices[k0 + 1]
                if k0 == KT - 2:
                    # final pair: both G halves stop BEFORE the last mains
                    # so the tail's DVE mask-multiplies run under them
                    g_mm(k0, x0)
                    g_mm(k0 + 1, x1)
                    main_mms(k0, x0, w0)
                    main_mms(k0 + 1, x1, w1)
                else:
                    main_mms(k0, x0, w0)
                    main_mms(k0 + 1, x1, w1)
                    g_mm(k0, x0)
                    g_mm(k0 + 1, x1)

            # ---- tail ----
            # mask-multiply each G column-half straight out of PSUM (one
            # PSUM input per DVE op), merge into ht (f16), then the four
            # f16 tail matmuls back-to-back; copies alternate DVE/ACT and
            # the output DMAs alternate sync/gpsimd so nothing serializes
            # on a single engine.
            o_all = opool.tile([P, BT * OUT_L], F16)
            # ht0 depends only on the even G half (stops at k=30, before
            # the final mains) so both mask-multiplies overlap the end of
            # the stream; the tail is then pure back-to-back matmuls
            nc.vector.tensor_tensor(
                out=ht0[0:NUM_CLASS, :], in0=gt0_ps[0:NUM_CLASS, :],
                in1=maskT, op=mybir.AluOpType.mult,
            )
            nc.vector.tensor_tensor(
                out=ht1, in0=gt1_ps[NUM_CLASS : 2 * NUM_CLASS, :],
                in1=maskT, op=mybir.AluOpType.mult,
            )
            for bt in range(BT):
                sl = slice(bt * P, (bt + 1) * P)
                nc.tensor.matmul(
                    mps[bt],
                    lhsT=ht0[:, sl],
                    rhs=bS_sb,
                    start=False,
                    stop=False,
                )
                nc.tensor.matmul(
                    mps[bt],
                    lhsT=ht1[:, sl],
                    rhs=bS_sb[0:NUM_CLASS, :],
                    start=False,
                    stop=True,
                )
            for bt in range(BT):
                sl = slice(bt * P, (bt + 1) * P)
                osl = o_all[:, bt * OUT_L : (bt + 1) * OUT_L]
                if bt % 2 == 0:
                    nc.vector.tensor_copy(out=osl, in_=mps[bt])
                    nc.sync.dma_start(out=out[sl, :], in_=osl)
                else:
                    nc.scalar.copy(out=osl, in_=mps[bt])
                    nc.scalar.dma_start(out=out[sl, :], in_=osl)

    nc.finalize()
    _cache[key] = nc
    return nc


def _pack_inputs(x, pseudo_index, weight, bias, lora_A, lora_B):
    """Build the interleaved per-core xw chunk buffers + replicated small
    inputs (lora_A chunks, pseudo_index layouts, scaled lora_B/bias)."""
    xT = np.ascontiguousarray(x.T).astype(np.float16)   # [IN, B]
    aT = lora_A[:NUM_CLASS].T.astype(np.float16)        # [IN, 64]

    # av: [128, AVW]: [p, k*64+c] = aT[k*128+p, c], then bS (per-core)
    av_base = np.zeros((P, AVW), dtype=np.float16)
    av_base[:, :AW] = (
        aT.reshape(KT, P, NUM_CLASS).transpose(1, 0, 2).reshape(P, KT * NUM_CLASS)
    )

    pp_base = np.zeros((P, PPW), dtype=np.float32)
    pp_base[:, PSOFF : PSOFF + BT * NUM_CLASS] = (
        pseudo_index.reshape(BT, P, NUM_CLASS)
        .transpose(1, 0, 2)
        .reshape(P, BT * NUM_CLASS)
    )
    pp_base[:NUM_CLASS, PTOFF : PTOFF + B] = pseudo_index.T

    # per-chunk x blocks: for chunk c with k-tiles [k0, k0+ck):
    #   [p, kk*B + b] = xT[(k0+kk)*P + p, b]
    x3 = xT.reshape(KT, P, B)  # [k, p, b]

    in_maps = []
    for i in range(NCORES):
        o0 = i * OUT_L
        wTi = weight[o0 : o0 + OUT_L].T.astype(np.float16)  # [IN, OUT_L]
        w3 = wTi.reshape(KT, P, OUT_L)
        xwi = np.zeros((NCHUNK, P, XW_WIDTH), dtype=np.float16)
        k0 = 0
        for c, ck in enumerate(CHUNK_KS):
            xwi[c, :, : ck * B] = (
                x3[k0 : k0 + ck].transpose(1, 0, 2).reshape(P, ck * B)
            )
            xwi[c, :, ck * B : ck * (B + OUT_L)] = (
                w3[k0 : k0 + ck].transpose(1, 0, 2).reshape(P, ck * OUT_L)
            )
            k0 += ck
        avi = av_base.copy()
        avi[:NUM_CLASS, AW:AVW] = 16.0 * lora_B[o0 : o0 + OUT_L, :NUM_CLASS].T
        avi[NUM_CLASS, AW:AVW] = 2.0 * bias[o0 : o0 + OUT_L]
        in_maps.append({"xw": xwi, "av": avi, "pp": pp_base})
    return in_maps


def kernel(x, pseudo_index, weight, bias, lora_A, lora_B):
    global last_results
    x = np.ascontiguousarray(np.asarray(x, dtype=np.float32))
    pseudo_index = np.ascontiguousarray(np.asarray(pseudo_index, dtype=np.float32))
    weight = np.asarray(weight, dtype=np.float32)
    bias = np.asarray(bias, dtype=np.float32)
    lora_A = np.asarray(lora_A, dtype=np.float32)
    lora_B = np.asarray(lora_B, dtype=np.float32)

    nc = _build()
    in_maps = _pack_inputs(x, pseudo_index, weight, bias, lora_A, lora_B)
    res = run_bass_kernel_spmd(nc, in_maps, list(range(NCORES)))
    last_results = res
    return np.hstack(
        [res.results[i]["out"].astype(np.float32) for i in range(NCORES)]
    )
